# revision 33
# baseline (speedup 1.0000x reference)
"""CV neural network (6 modes, cutoff 3, 6 layers) on 8 trn2 NeuronCores.

Algebra: the reference circuit is
    psi0(x_b) = kron_m expm(x_bm * D_GEN)[:, 0]          (closed form, host)
    psi       = C @ psi0                                  (C fixed 729x729)
    out[b,m]  = Re( psi^H (I (x) X_OP (x) I) psi )        (host)
Everything between the data-encoding displacements and the expectations is a
fixed linear operator C on the 729-dim truncated Fock space, depending only on
the (tiny) layer parameters.  The host folds the circuit into UT = C^T once
(complex128), and the device does the only heavy part: the complex matmul
psi[b, i] = sum_j psi0[b, j] * UT[j, i] for 1024 batch samples.

Sharding: batch 4-way x output-column (i) 2-way = 8 cores.  Per core:
  p_re/p_im: [729, 256]  psi0^T batch-quarter (j rows, b cols)
  u_re/u_im: [729, 365]  UT column half (half 1 overlaps one column)
  o_re/o_im: [256, 365]  psi shard (b rows, i cols)
Complex matmul via 4 real matmuls; p_im is negated on-device so both psum
groups are pure '+' accumulations and outputs DMA straight from PSUM.
"""
import os
import numpy as np

N_MODES, N_LAYERS, CUTOFF, BATCH = 6, 6, 3, 1024
M2 = N_MODES * (N_MODES - 1) // 2
DIM = CUTOFF ** N_MODES                      # 729
N_CORES = 8
B_SHARD = BATCH // 4                         # 256 (batch quarter)
I_SHARD = 366                                # even (fp32r needs even N); overlap 3
I_START = (0, DIM - I_SHARD)                 # (0, 363)
DIM_PAD = 768                                # 6 x 128 (rows 729.. are zero)
NJ = 6                                       # j tiles, all K=128 after padding

MM_F32R = True  # float32r matmul inputs: 1 cyc/row vs fp32's 4 (N>=256)

# Results of the last device run (for the test harness to inspect).
LAST_RESULT = None

# ----------------------------------------------------------------- host math

_a = np.diag(np.sqrt(np.arange(1, CUTOFF)), 1).astype(np.complex128)
_ad = _a.conj().T
_NVEC = np.arange(CUTOFF, dtype=np.float64)
_X_OP = (_a + _ad).real
_BS_GEN = np.kron(_ad, _a) - np.kron(_a, _ad)
_SQ_GEN = _a @ _a - _ad @ _ad
_D_GEN = _ad - _a


def _expm_factory(G):
    """G anti-Hermitian. Returns f(t) = expm(t*G), vectorized over real t."""
    lam, V = np.linalg.eigh(1j * G)
    Vh = V.conj().T

    def f(t):
        t = np.asarray(t, dtype=np.float64)
        ph = np.exp(-1j * np.multiply.outer(t, lam))
        return np.einsum('ij,...j,jk->...ik', V, ph, Vh)
    return f


_disp_gate = _expm_factory(_D_GEN)
_sq_gate_half = _expm_factory(0.5 * _SQ_GEN)
_bs_gate = _expm_factory(_BS_GEN)


def _apply_1(psi, U, m):
    psi = np.moveaxis(psi, 1 + m, -1)
    psi = psi @ U.T
    return np.moveaxis(psi, -1, 1 + m)


def _apply_2(psi, U, m):
    psi = np.moveaxis(psi, (1 + m, 2 + m), (-2, -1))
    sh = psi.shape
    psi = (psi.reshape(sh[:-2] + (CUTOFF * CUTOFF,)) @ U.T).reshape(sh)
    return np.moveaxis(psi, (-2, -1), (1 + m, 2 + m))


def _apply_diag(psi, d, m):
    shape = [1] * psi.ndim
    shape[1 + m] = CUTOFF
    return psi * d.reshape(shape)


def _interferometer(psi, params):
    theta = params[:M2]
    rphi = params[-N_MODES:]
    n = 0
    for l in range(N_MODES):
        for k in range(N_MODES - 1):
            if (l + k) % 2 != 1:
                psi = _apply_2(psi, _bs_gate(theta[n]), k)
                n += 1
    for i in range(max(1, N_MODES - 1)):
        psi = _apply_diag(psi, np.exp(1j * rphi[i] * _NVEC), i)
    return psi


def _build_UT(theta_1, theta_2, squeezing_r, displacement_r, kerr_params):
    """UT[j, i] = C[i, j]: apply the post-encoding circuit to basis vectors."""
    psi = np.eye(DIM, dtype=np.complex128).reshape((DIM,) + (CUTOFF,) * N_MODES)
    for L in range(N_LAYERS):
        psi = _interferometer(psi, theta_1[L])
        for m in range(N_MODES):
            psi = _apply_1(psi, _sq_gate_half(squeezing_r[L, m] * 0.5), m)
        psi = _interferometer(psi, theta_2[L])
        for m in range(N_MODES):
            psi = _apply_1(psi, _disp_gate(displacement_r[L, m]), m)
            psi = _apply_diag(
                psi, np.exp(1j * (kerr_params[L, m] * 0.001) * _NVEC * _NVEC), m)
    return psi.reshape(DIM, DIM)


def _build_psi0(x):
    """x: (B, 6) -> flattened kron of displacement columns, (B, 729)."""
    v = _disp_gate(x)[..., :, 0]
    out = v[:, 0, :]
    for m in range(1, N_MODES):
        out = np.einsum('bi,bj->bij', out, v[:, m, :]).reshape(x.shape[0], -1)
    return out


def _expectation(psi_flat):
    """psi_flat: (B, 729) complex -> (B, 6) float64: <X_m>."""
    B = psi_flat.shape[0]
    outs = []
    for m in range(N_MODES):
        pre, post = CUTOFF ** m, CUTOFF ** (N_MODES - 1 - m)
        psi = psi_flat.reshape(B, pre, CUTOFF, post)
        phi = np.einsum('ij,bpjq->bpiq', _X_OP, psi)
        outs.append(np.sum(psi.conj() * phi, axis=(1, 2, 3)).real)
    return np.stack(outs, axis=1)


# --------------------------------------------------------------- bass kernel

def _build_bass():
    import concourse.mybir as mybir
    import concourse.tile as tile
    from concourse import bacc

    nc = bacc.Bacc("TRN2", target_bir_lowering=False, debug=False,
                   enable_asserts=False, num_devices=N_CORES)
    f32 = mybir.dt.float32
    mdt = mybir.dt.float32r if MM_F32R else f32

    bf16 = mybir.dt.bfloat16
    # Host-pre-tiled inputs, one DMA per (tensor, j-block):
    #   u_ri: [256, 3*(2*I_SHARD)]  row = 128*h + p; block jt=3h+a at cols
    #         a*732, [0:366]=re, [366:732]=im
    #   p_ri: [256, 3*(2*B_SHARD)]  likewise, [0:256]=re, [256:512]=im
    # Loaded straight into float32r tiles (fp32r's precision loss happens in
    # the PE; DMA moves bits unchanged).
    UB, PB = 2 * I_SHARD, 2 * B_SHARD
    u_ri = nc.dram_tensor("u_ri", [256, 3 * UB], mdt, kind="ExternalInput").ap()
    p_ri = nc.dram_tensor("p_ri", [256, 3 * PB], mdt, kind="ExternalInput").ap()
    o_ri = nc.dram_tensor("o_ri", [B_SHARD, 2 * I_SHARD], bf16,
                          kind="ExternalOutput").ap()

    with tile.TileContext(nc) as tc:
        with (
            tc.tile_pool(name="u", bufs=6) as u_pool,
            tc.tile_pool(name="p", bufs=6) as p_pool,
            tc.tile_pool(name="ps", bufs=2, space="PSUM") as ps_pool,
            tc.tile_pool(name="o", bufs=2) as o_pool,
            tc.tile_pool(name="s", bufs=1) as s_pool,
        ):
            # PE warm-up: dummy f32r matmuls bridge the input-load window so
            # the HAM un-throttles (1.2 -> 2.4 GHz) before real matmuls.
            wsrc0 = s_pool.tile([128, 640], f32, tag="warm0", name="warm0")
            nc.vector.memset(wsrc0[:, :], 0)
            wsrc = s_pool.tile([128, 640], mdt, tag="warm", name="warm")
            nc.vector.tensor_copy(out=wsrc, in_=wsrc0)
            ps_w = ps_pool.tile([128, 512], f32, tag="psw", name="psw", bufs=1)
            for w in range(10):
                nc.tensor.matmul(ps_w, wsrc[:, 0:128], wsrc[:, 128:640],
                                 start=True, stop=True)

            # Per-block loads in consumption (jt) order: U alternates between
            # the two HWDGE rings, P rides the SWDGE ring, so block jt's
            # operands land together and matmuls trail block arrivals.
            u_t, p_t, pn = {}, {}, {}
            for jt in range(NJ):
                h, a = divmod(jt, 3)
                kj = 128
                rs = slice(h * 128, h * 128 + kj)
                # jt 0-2 ride the lag-free HWDGE rings (SWDGE has a ~4us
                # start lag); jt 3-5 ride SWDGE, interleaved u/p per block.
                u_t[jt] = u_pool.tile([128, UB], mdt, tag="u", name=f"u{jt}")
                p_t[jt] = p_pool.tile([128, PB], mdt, tag="p", name=f"p{jt}")
                u_eng = nc.scalar if jt < 3 else nc.gpsimd
                p_eng = nc.sync if jt in (0, 1, 2, 4, 5) else nc.gpsimd
                u_eng.dma_start(out=u_t[jt][:kj], in_=u_ri[rs, a * UB:(a + 1) * UB])
                p_eng.dma_start(out=p_t[jt][:kj], in_=p_ri[rs, a * PB:(a + 1) * PB])
                pn[jt] = p_pool.tile([128, B_SHARD], mdt, tag="pn", name=f"pn{jt}")
                nc.vector.tensor_scalar_mul(
                    pn[jt][:kj], p_t[jt][:kj, B_SHARD:].bitcast(f32), -1.0)

            ps_re, ps_im = {}, {}
            for bt in range(2):
                ps_re[bt] = ps_pool.tile([128, I_SHARD], f32, tag="psre",
                                         name=f"psre{bt}")
                ps_im[bt] = ps_pool.tile([128, I_SHARD], f32, tag="psim",
                                         name=f"psim{bt}")
            for jt in range(NJ):
                kj = 89 if jt == NJ - 1 else 128
                ur = u_t[jt][:kj, :I_SHARD]
                ui = u_t[jt][:kj, I_SHARD:]
                for bt in range(2):
                    bs = slice(bt * 128, (bt + 1) * 128)
                    first, last = jt == 0, jt == NJ - 1
                    # re = Pr.Ur + (-Pi).Ui ; im = Pr.Ui + Pi.Ur
                    nc.tensor.matmul(ps_re[bt], p_t[jt][:kj, bs], ur,
                                     start=first, stop=False)
                    nc.tensor.matmul(ps_im[bt], p_t[jt][:kj, bs], ui,
                                     start=first, stop=False)
                    nc.tensor.matmul(ps_re[bt], pn[jt][:kj, bs], ui,
                                     start=False, stop=last)
                    nc.tensor.matmul(ps_im[bt],
                                     p_t[jt][:kj, B_SHARD:][:, bs], ur,
                                     start=False, stop=last)
            for bt in range(2):
                bs_o = slice(bt * 128, (bt + 1) * 128)
                sb = o_pool.tile([128, 2 * I_SHARD], bf16, tag="sb",
                                 name=f"sb{bt}")
                nc.vector.tensor_copy(out=sb[:, :I_SHARD], in_=ps_re[bt])
                nc.scalar.copy(out=sb[:, I_SHARD:], in_=ps_im[bt])
                (nc.sync if bt == 0 else nc.scalar).dma_start(
                    out=o_ri[bs_o], in_=sb)
    nc.compile()
    return nc


def kernel(x, theta_1, theta_2, squeezing_r, displacement_r, kerr_params):
    global LAST_RESULT
    x = np.asarray(x, dtype=np.float32)
    UT = _build_UT(np.asarray(theta_1, np.float64), np.asarray(theta_2, np.float64),
                   np.asarray(squeezing_r, np.float64),
                   np.asarray(displacement_r, np.float64),
                   np.asarray(kerr_params, np.float64))
    psi0 = _build_psi0(x.astype(np.float64))          # (B, 729) complex128
    p_t = psi0.T                                      # (729, B)

    UT_pad = np.zeros((DIM_PAD, DIM), np.complex128)
    UT_pad[:DIM] = UT
    p_pad = np.zeros((DIM_PAD, BATCH), np.complex128)
    p_pad[:DIM] = p_t

    def pack_ri(arr):
        """[768, W] complex -> [256, 3*2W]: rows 128h+p; block a at cols
        a*2W with [0:W]=re, [W:2W]=im."""
        w = arr.shape[1]
        out = np.empty((2, 128, 3, 2 * w), np.float32)
        blk = arr.reshape(2, 3, 128, w)
        out[:, :, :, :w] = blk.real.transpose(0, 2, 1, 3)
        out[:, :, :, w:] = blk.imag.transpose(0, 2, 1, 3)
        return np.ascontiguousarray(out.reshape(256, 6 * w))

    in_maps = []
    for c in range(N_CORES):
        q, h = divmod(c, 2)
        bsl = slice(q * B_SHARD, (q + 1) * B_SHARD)
        isl = slice(I_START[h], I_START[h] + I_SHARD)
        in_maps.append({
            "u_ri": pack_ri(UT_pad[:, isl]),
            "p_ri": pack_ri(p_pad[:, bsl]),
        })

    # bass_utils' trace path does `from antenv.axon_hooks import ...`
    # unguarded; this image's antenv lacks that module.  Provide a stub so
    # tracing degrades gracefully instead of crashing (e.g. if BASS_TRACE=1).
    try:
        import antenv.axon_hooks  # noqa: F401
    except ImportError:
        import sys
        import types
        stub = types.ModuleType("antenv.axon_hooks")
        stub._hook = None
        stub.set_axon_ntff_profile_hook = lambda h: setattr(stub, "_hook", h)
        stub.get_axon_ntff_profile_hook = lambda: stub._hook
        sys.modules["antenv.axon_hooks"] = stub

    from concourse.bass_utils import run_bass_kernel_spmd
    nc = _build_bass()
    res = run_bass_kernel_spmd(nc, in_maps, core_ids=list(range(N_CORES)),
                               trace=bool(int(os.environ.get("KERNEL_TRACE", "0"))))
    LAST_RESULT = res

    psi = np.empty((BATCH, DIM), dtype=np.complex128)
    for c in range(N_CORES):
        q, h = divmod(c, 2)
        o = res.results[c]["o_ri"]
        sh = (o[:, :I_SHARD].astype(np.float64)
              + 1j * o[:, I_SHARD:].astype(np.float64))
        bsl = slice(q * B_SHARD, (q + 1) * B_SHARD)
        if h == 0:
            psi[bsl, 0:I_SHARD] = sh
        else:
            psi[bsl, I_SHARD:DIM] = sh[:, I_SHARD - (DIM - I_SHARD):]
    return _expectation(psi).astype(np.float32)
